# revision 1
# baseline (speedup 1.0000x reference)
"""Trainium2 Bass kernel for 4D cubic B-spline grid evaluation.

Problem: for each of 65536 query coords u in [0,1)^4, evaluate a uniform cubic
B-spline over an (8,16,16,16) control grid with 32 channels and linear-
extrapolation padding -> output (65536, 32) f32.

Strategy (data-parallel over the query batch, 8 cores x 8192 queries):
  * The linear-extrapolation grid padding is folded into transformed boundary
    weights, so no padded grid is ever materialized.
  * The grid is re-laid once in DRAM as a w-unfolded table: unit (t,d,h,wc) =
    the 4-wide w-window starting at clamped cell wc, i.e. 4x32ch = 512B
    contiguous.  26624 units, so indices fit dma_gather's int16.
  * Per query the other three dims contribute 4x4x4 = 64 units, gathered with
    SWDGE dma_gather (512B descriptors run at full DMA-bus rate; 8 sub-gathers
    of 1024 indices per 128-query tile since the descriptor ring holds 1024).
  * Separable weighted reduction (w, then h, d, t): the Scalar engine (ACT)
    computes the per-partition-scalar products of the two big stages (it owns
    the strided reads), the Vector engine (DVE) does contiguous adds plus the
    small-stage FMA chains, so the whole reduction hides under the gather DMA.
"""

import numpy as np

import concourse.bacc as bacc
import concourse.bass as bass
import concourse.mybir as mybir
import concourse.tile as tile
from concourse.bass_utils import run_bass_kernel_spmd

P = 128              # partitions / queries per tile
NT = 64              # tiles per core
BSH = P * NT         # 8192 queries per core
NCORES = 8
B = BSH * NCORES     # 65536
C = 32
SIZES = (8, 16, 16, 16)          # t, d, h, w control-point resolution
WCELLS = 13                      # distinct clamped w-window starts (0..12)
UNIT = 4 * 32                    # one gather unit: 4 w-points x 32 ch
NUNITS = 8 * 16 * 16 * WCELLS    # 26624 (< 32767, fits int16 indices)
F32 = mybir.dt.float32
I32 = mybir.dt.int32
I16 = mybir.dt.int16

_CACHED_NC = None


def _cubic_weights(nc, pool, f, nt):
    """Emit DVE ops computing the 4 cubic B-spline weights of fractional
    position tile `f` ([P, nt] f32).  Returns 4 tiles [P, nt]."""
    v = nc.vector
    A = mybir.AluOpType
    f2 = pool.tile([P, nt], F32, tag="f2")
    f3 = pool.tile([P, nt], F32, tag="f3")
    v.tensor_tensor(out=f2[:], in0=f[:], in1=f[:], op=A.mult)
    v.tensor_tensor(out=f3[:], in0=f2[:], in1=f[:], op=A.mult)
    w0 = pool.tile([P, nt], F32, tag="w0")
    w1 = pool.tile([P, nt], F32, tag="w1")
    w2 = pool.tile([P, nt], F32, tag="w2")
    w3 = pool.tile([P, nt], F32, tag="w3")
    tmp = pool.tile([P, nt], F32, tag="wtmp")
    # w0 = (1-f)^3/6 = -(f-1)^3/6
    v.tensor_scalar(out=tmp[:], in0=f[:], scalar1=1.0, scalar2=None, op0=A.subtract)
    v.tensor_tensor(out=w0[:], in0=tmp[:], in1=tmp[:], op=A.mult)
    v.tensor_tensor(out=w0[:], in0=w0[:], in1=tmp[:], op=A.mult)
    v.tensor_scalar(out=w0[:], in0=w0[:], scalar1=-1.0 / 6.0, scalar2=None, op0=A.mult)
    # w1 = 2/3 - f2 + f3/2  ->  (f3*0.5 - f2) + 2/3
    v.scalar_tensor_tensor(out=w1[:], in0=f3[:], scalar=0.5, in1=f2[:],
                           op0=A.mult, op1=A.subtract)
    v.tensor_scalar(out=w1[:], in0=w1[:], scalar1=2.0 / 3.0, scalar2=None, op0=A.add)
    # w2 = 1/6 + (f + f2 - f3)/2
    v.tensor_tensor(out=w2[:], in0=f[:], in1=f2[:], op=A.add)
    v.tensor_tensor(out=w2[:], in0=w2[:], in1=f3[:], op=A.subtract)
    v.tensor_scalar(out=w2[:], in0=w2[:], scalar1=0.5, scalar2=1.0 / 6.0,
                    op0=A.mult, op1=A.add)
    # w3 = f3/6
    v.tensor_scalar(out=w3[:], in0=f3[:], scalar1=1.0 / 6.0, scalar2=None, op0=A.mult)
    return w0, w1, w2, w3


def _build_nc(no_reduce=False, no_gather=False):
    nc = bacc.Bacc("TRN2", target_bir_lowering=False, debug=False,
                   num_devices=NCORES)
    u_in = nc.dram_tensor("u", [BSH, 4], F32, kind="ExternalInput")
    g_in = nc.dram_tensor("grid", [SIZES[0] * SIZES[1] * SIZES[2] * SIZES[3], C],
                          F32, kind="ExternalInput")
    out = nc.dram_tensor("out", [BSH, C], F32, kind="ExternalOutput")
    utab = nc.dram_tensor("utab", [NUNITS, UNIT], F32)
    bscr = nc.dram_tensor("bscr", [BSH], I16)

    v = nc.vector
    A = mybir.AluOpType

    with tile.TileContext(nc) as tc:
        with (
            tc.tile_pool(name="persist", bufs=1) as pp,
            tc.tile_pool(name="scratch", bufs=2) as sp,
            tc.tile_pool(name="gather", bufs=3) as gp,
            tc.tile_pool(name="red", bufs=2) as rp,
            tc.tile_pool(name="prod", bufs=3) as prp,
        ):
            # ---------------- Phase A: per-query prep for the whole shard ---
            U = pp.tile([P, NT, 4], F32)
            # query q = t*128 + p  ->  partition p, slot t
            nc.sync.dma_start(
                out=U[:], in_=u_in[:].rearrange("(t p) d -> p t d", p=P))

            # per-dim transformed weights [P, NT, 4] and window starts [P, NT]
            Wd_tiles = []
            O_tiles = []
            for dim in range(4):
                n = float(SIZES[dim])
                s = sp.tile([P, NT], F32, tag="s")
                v.tensor_scalar(out=s[:], in0=U[:, :, dim], scalar1=n - 1.0,
                                scalar2=None, op0=A.mult)
                # floor(s) via int cast round-trip; i = r - (s < r) is correct
                # whether the f32->i32 cast truncates or rounds-to-nearest
                # (s >= 0 always here).
                ri = sp.tile([P, NT], I32, tag="ri")
                v.tensor_copy(out=ri[:], in_=s[:])
                rf = sp.tile([P, NT], F32, tag="rf")
                v.tensor_copy(out=rf[:], in_=ri[:])
                flt = sp.tile([P, NT], F32, tag="flt")
                v.tensor_tensor(out=flt[:], in0=s[:], in1=rf[:], op=A.is_lt)
                ifl = sp.tile([P, NT], F32, tag="ifl")
                v.tensor_tensor(out=ifl[:], in0=rf[:], in1=flt[:], op=A.subtract)
                ic = sp.tile([P, NT], F32, tag="ic")
                v.tensor_scalar(out=ic[:], in0=ifl[:], scalar1=n - 2.0,
                                scalar2=None, op0=A.min)
                f = sp.tile([P, NT], F32, tag="f")
                v.tensor_tensor(out=f[:], in0=s[:], in1=ic[:], op=A.subtract)
                mL = sp.tile([P, NT], F32, tag="mL")
                v.tensor_scalar(out=mL[:], in0=ic[:], scalar1=0.0, scalar2=None,
                                op0=A.is_equal)
                mR = sp.tile([P, NT], F32, tag="mR")
                v.tensor_scalar(out=mR[:], in0=ic[:], scalar1=n - 2.0,
                                scalar2=None, op0=A.is_equal)
                # window start o = clip(i-1, 0, n-4)
                O = pp.tile([P, NT], F32, tag=f"O{dim}")
                v.tensor_scalar(out=O[:], in0=ic[:], scalar1=1.0, scalar2=0.0,
                                op0=A.subtract, op1=A.max)
                v.tensor_scalar(out=O[:], in0=O[:], scalar1=n - 4.0,
                                scalar2=None, op0=A.min)
                O_tiles.append(O)

                w0, w1, w2, w3 = _cubic_weights(nc, sp, f, NT)
                # boundary delta vectors:
                #   left  (i==0):   wL = (w1+2w0, w2-w0, w3, 0)
                #   right (i==n-2): wR = (0, w0, w1-w3, w2+2w3)
                # w' = w + mL*(wL-w) + mR*(wR-w)
                WT = pp.tile([P, NT, 4], F32, tag=f"W{dim}")
                dl = sp.tile([P, NT], F32, tag="dl")
                dr = sp.tile([P, NT], F32, tag="dr")
                acc = sp.tile([P, NT], F32, tag="wacc")

                # component 0: dL0 = w0+w1, dR0 = -w0
                v.tensor_tensor(out=dl[:], in0=w0[:], in1=w1[:], op=A.add)
                v.tensor_tensor(out=dl[:], in0=dl[:], in1=mL[:], op=A.mult)
                v.tensor_tensor(out=dr[:], in0=w0[:], in1=mR[:], op=A.mult)
                v.tensor_tensor(out=acc[:], in0=w0[:], in1=dl[:], op=A.add)
                v.tensor_tensor(out=WT[:, :, 0], in0=acc[:], in1=dr[:],
                                op=A.subtract)
                # component 1: dL1 = w2-w0-w1, dR1 = w0-w1
                v.tensor_tensor(out=dl[:], in0=w2[:], in1=w0[:], op=A.subtract)
                v.tensor_tensor(out=dl[:], in0=dl[:], in1=w1[:], op=A.subtract)
                v.tensor_tensor(out=dl[:], in0=dl[:], in1=mL[:], op=A.mult)
                v.tensor_tensor(out=dr[:], in0=w0[:], in1=w1[:], op=A.subtract)
                v.tensor_tensor(out=dr[:], in0=dr[:], in1=mR[:], op=A.mult)
                v.tensor_tensor(out=acc[:], in0=w1[:], in1=dl[:], op=A.add)
                v.tensor_tensor(out=WT[:, :, 1], in0=acc[:], in1=dr[:], op=A.add)
                # component 2: dL2 = w3-w2, dR2 = w1-w2-w3
                v.tensor_tensor(out=dl[:], in0=w3[:], in1=w2[:], op=A.subtract)
                v.tensor_tensor(out=dl[:], in0=dl[:], in1=mL[:], op=A.mult)
                v.tensor_tensor(out=dr[:], in0=w1[:], in1=w2[:], op=A.subtract)
                v.tensor_tensor(out=dr[:], in0=dr[:], in1=w3[:], op=A.subtract)
                v.tensor_tensor(out=dr[:], in0=dr[:], in1=mR[:], op=A.mult)
                v.tensor_tensor(out=acc[:], in0=w2[:], in1=dl[:], op=A.add)
                v.tensor_tensor(out=WT[:, :, 2], in0=acc[:], in1=dr[:], op=A.add)
                # component 3: dL3 = -w3, dR3 = w2+w3
                v.tensor_tensor(out=dl[:], in0=w3[:], in1=mL[:], op=A.mult)
                v.tensor_tensor(out=dr[:], in0=w2[:], in1=w3[:], op=A.add)
                v.tensor_tensor(out=dr[:], in0=dr[:], in1=mR[:], op=A.mult)
                v.tensor_tensor(out=acc[:], in0=w3[:], in1=dl[:], op=A.subtract)
                v.tensor_tensor(out=WT[:, :, 3], in0=acc[:], in1=dr[:], op=A.add)
                Wd_tiles.append(WT)

            # ---- w-unfolded gather table: unit (t,d,h,wc) = 4 w-points x
            # 32 ch = 128 contiguous f32 (512B).  26624 units fits int16.
            for wc in range(WCELLS):
                nc.sync.dma_start(
                    out=utab[:].rearrange("(g w) e -> g w e", w=WCELLS)[:, wc, :],
                    in_=g_in[:].rearrange("(g x) c -> g (x c)", x=SIZES[3])[
                        :, wc * C : wc * C + UNIT],
                )

            # unit base index = ((ot*16+od)*16+oh)*WCELLS+ow, as int16
            base_f = pp.tile([P, NT], F32)
            v.scalar_tensor_tensor(out=base_f[:], in0=O_tiles[0][:], scalar=16.0,
                                   in1=O_tiles[1][:], op0=A.mult, op1=A.add)
            v.scalar_tensor_tensor(out=base_f[:], in0=base_f[:], scalar=16.0,
                                   in1=O_tiles[2][:], op0=A.mult, op1=A.add)
            v.scalar_tensor_tensor(out=base_f[:], in0=base_f[:],
                                   scalar=float(WCELLS), in1=O_tiles[3][:],
                                   op0=A.mult, op1=A.add)
            base_i = pp.tile([P, NT], I16)
            v.tensor_copy(out=base_i[:], in_=base_f[:])
            # bounce to DRAM in query order so per-tile loads can re-wrap it
            # into dma_gather's 16-partition index layout
            nc.sync.dma_start(
                out=bscr[:].rearrange("(t p) -> p t", p=P), in_=base_i[:])

            # window offsets i*(16*16*WCELLS) + j*(16*WCELLS) + k*WCELLS,
            # (i,j,k) C-order, replicated on all partitions
            offs = pp.tile([P, 64], I16)
            nc.gpsimd.iota(
                out=offs[:],
                pattern=[[256 * WCELLS, 4], [16 * WCELLS, 4], [WCELLS, 4]],
                base=0, channel_multiplier=0)

            # re-load bases wrapped for dma_gather's index layout:
            # bwall[p', t, jj] = base[query t*128 + jj*16 + p'%16], i.e. each
            # 16-partition Q7 group holds a replica (8 replication DMAs).
            bwall = pp.tile([P, NT, 8], I16)
            for g2 in range(8):
                nc.sync.dma_start(
                    out=bwall[g2 * 16 : (g2 + 1) * 16, :, :],
                    in_=bass.AP(bscr, 0, [[1, 16], [128, NT], [16, 8]]),
                )

            # ---------------- Phase B: per-tile gather + reduce ------------
            # ACT (ScalarE) can only do out = in*scale (per-partition scalar),
            # so it takes pure products; DVE (VectorE) does the FMA/add chain.
            wt, wd, wh, ww = Wd_tiles
            Copy = mybir.ActivationFunctionType.Copy
            for t in range(NT):
                # wrapped index layout: idx[p, w*8+jj] = base[q=jj*16+p%16]
                # + offs[w]; dma_gather reads list pos n at [n%16, n//16] and
                # writes gather n to partition n%128, slot n//128.
                idx = sp.tile([P, 64, 8], I16, tag="idx")
                v.tensor_tensor(
                    out=idx[:],
                    in0=bwall[:, t : t + 1, :].to_broadcast([P, 64, 8]),
                    in1=offs[:].rearrange("p (w o) -> p w o", o=1).to_broadcast(
                        [P, 64, 8]),
                    op=A.add,
                )
                g = gp.tile([P, 64, UNIT], F32, tag="g")
                # the SWDGE descriptor ring holds 1024 descriptors, so split
                # the tile's 8192-window gather into 8 sub-gathers
                for k in (range(0) if no_gather else range(8)):
                    nc.gpsimd.dma_gather(
                        out_ap=g[:, 8 * k : 8 * (k + 1), :],
                        in_ap=utab[:],
                        idxs_ap=idx[:, 8 * k : 8 * (k + 1), :].rearrange(
                            "p w j -> p (w j)"),
                        num_idxs=P * 8,
                        num_idxs_reg=P * 8,
                        elem_size=UNIT,
                    )

                if no_reduce:
                    otile0 = rp.tile([P, C], F32, tag="otile")
                    v.tensor_copy(out=otile0[:], in_=g[:, 0, 0:C])
                    nc.sync.dma_start(out=out[t * P : (t + 1) * P, :],
                                      in_=otile0[:])
                    continue

                def xsl(l):  # gathered l-slice [P, 64, C]
                    return g[:, :, l * C : (l + 1) * C]

                # ACT computes all weighted products (it owns the strided
                # reads); DVE does only contiguous adds.  Per-stage product
                # tiles live in the prod pool (bufs=3) for overlap.
                def stage(src_fn, wtile, nwin, ptag, stag):
                    prods = []
                    for l in range(4):
                        pl = prp.tile([P, nwin, C], F32, tag=ptag)
                        nc.scalar.activation(out=pl[:], in_=src_fn(l),
                                             func=Copy,
                                             scale=wtile[:, t, l : l + 1])
                        prods.append(pl)
                    s0 = rp.tile([P, nwin, C], F32, tag=stag + "0")
                    s1 = rp.tile([P, nwin, C], F32, tag=stag + "1")
                    v.tensor_tensor(out=s0[:], in0=prods[0][:],
                                    in1=prods[1][:], op=A.add)
                    v.tensor_tensor(out=s1[:], in0=s0[:], in1=prods[2][:],
                                    op=A.add)
                    s2 = rp.tile([P, nwin, C], F32, tag=stag + "0")
                    v.tensor_tensor(out=s2[:], in0=s1[:], in1=prods[3][:],
                                    op=A.add)
                    return s2

                # stage w (l): y[win, c] = sum_l g[win, l, c]*ww_l
                y = stage(xsl, ww, 64, "pw", "yw")
                yv = y[:].rearrange("p (ij k) c -> p ij k c", k=4)
                # stage h (k): z[ij, c] = sum_k y[ij, k, c]*wh_k
                z = stage(lambda k: yv[:, :, k, :], wh, 16, "ph", "zh")
                zv = z[:].rearrange("p (i j) c -> p i j c", j=4)
                # stage d (j): small -> DVE FMA chain
                d0 = rp.tile([P, 4, C], F32, tag="dd0")
                d1 = rp.tile([P, 4, C], F32, tag="dd1")
                v.tensor_scalar(out=d0[:], in0=zv[:, :, 0, :],
                                scalar1=wd[:, t, 0:1], scalar2=None, op0=A.mult)
                v.scalar_tensor_tensor(out=d1[:], in0=zv[:, :, 1, :],
                                       scalar=wd[:, t, 1:2], in1=d0[:],
                                       op0=A.mult, op1=A.add)
                v.scalar_tensor_tensor(out=d0[:], in0=zv[:, :, 2, :],
                                       scalar=wd[:, t, 2:3], in1=d1[:],
                                       op0=A.mult, op1=A.add)
                v.scalar_tensor_tensor(out=d1[:], in0=zv[:, :, 3, :],
                                       scalar=wd[:, t, 3:4], in1=d0[:],
                                       op0=A.mult, op1=A.add)
                dv = d1[:]
                # stage t (i): small -> DVE FMA chain
                o0 = rp.tile([P, C], F32, tag="oo0")
                o1 = rp.tile([P, C], F32, tag="oo1")
                v.tensor_scalar(out=o0[:], in0=dv[:, 0, :],
                                scalar1=wt[:, t, 0:1], scalar2=None, op0=A.mult)
                v.scalar_tensor_tensor(out=o1[:], in0=dv[:, 1, :],
                                       scalar=wt[:, t, 1:2], in1=o0[:],
                                       op0=A.mult, op1=A.add)
                v.scalar_tensor_tensor(out=o0[:], in0=dv[:, 2, :],
                                       scalar=wt[:, t, 2:3], in1=o1[:],
                                       op0=A.mult, op1=A.add)
                o2 = rp.tile([P, C], F32, tag="oo2")
                v.scalar_tensor_tensor(out=o2[:], in0=dv[:, 3, :],
                                       scalar=wt[:, t, 3:4], in1=o0[:],
                                       op0=A.mult, op1=A.add)
                nc.sync.dma_start(out=out[t * P : (t + 1) * P, :], in_=o2[:])

    nc.compile()
    return nc


def _get_nc():
    global _CACHED_NC
    if _CACHED_NC is None:
        _CACHED_NC = _build_nc()
    return _CACHED_NC


def kernel(u: np.ndarray, grid: np.ndarray) -> np.ndarray:
    u = np.ascontiguousarray(np.asarray(u, dtype=np.float32))
    grid = np.ascontiguousarray(np.asarray(grid, dtype=np.float32))
    gflat = grid.reshape(-1, C)
    nc = _get_nc()
    in_maps = [
        {"u": u[c * BSH : (c + 1) * BSH], "grid": gflat}
        for c in range(NCORES)
    ]
    res = run_bass_kernel_spmd(nc, in_maps, list(range(NCORES)))
    return np.concatenate([res.results[c]["out"] for c in range(NCORES)], axis=0)


if __name__ == "__main__":
    rng = np.random.default_rng(0)
    u = rng.random((B, 4), dtype=np.float32)
    grid = rng.standard_normal((*SIZES, C), dtype=np.float32)
    out = kernel(u, grid)
    print(out.shape, out.dtype)



# revision 9
# speedup vs baseline: 1.8398x; 1.8398x over previous
"""Trainium2 Bass kernel for 4D cubic B-spline grid evaluation.

Problem: for each of 65536 query coords u in [0,1)^4, evaluate a uniform cubic
B-spline over an (8,16,16,16) control grid with 32 channels and linear-
extrapolation padding -> output (65536, 32) f32.

Strategy (data-parallel over the query batch, 8 cores x 8192 queries):
  * Linear-extrapolation padding is folded into transformed boundary weights
    (no padded grid is materialized).
  * The grid is re-laid once in DRAM as an fp16 table with 256B rows
    [t, d, wc, h] -> [4 w-points x 32 ch].  A gather element is 1024B spanning
    4 consecutive h-rows (elem_step=128 els < elem_size=512 els), so one
    descriptor fetches a full [4h, 4w, 32c] window pencil at >=512B, i.e. at
    full DMA-bus rate while fp16 halves the gathered bytes (16KB/query).
  * Per query only 4x4 (t,d) descriptors remain: 16 units land on partitions
    p=(qhat, s) (8 queries x 16 (t,d)-slots per 128-partition group).
  * The 16-slot (t,d)-contraction runs on the TensorEngine: 16 accumulating
    matmuls per 128-query tile with block-diagonal stationaries carrying the
    per-query wt*wd weights; PSUM ends up [128 q, 4h*4w*32c].
  * Per-query weight layout transforms (q-major -> (qhat,s)-major) are done by
    a tiny PE broadcast-matmul instead of DMA bounces.
  * DVE finishes with per-partition-scalar FMA chains over h then w.
"""

import numpy as np

import concourse.bacc as bacc
import concourse.bass as bass
import concourse.mybir as mybir
import concourse.tile as tile
from concourse.bass_utils import run_bass_kernel_spmd

P = 128              # partitions / queries per tile
NT = 64              # tiles per core
BSH = P * NT         # 8192 queries per core
NCORES = 8
B = BSH * NCORES     # 65536
C = 32
SIZES = (8, 16, 16, 16)          # t, d, h, w control-point resolution
T, D, H, W = SIZES
WC = 13                          # distinct clamped w-window starts (0..12)
ROW = 4 * C                      # table row: 4 w-points x 32 ch fp16 = 256B
UNIT = 4 * ROW                   # gather element: 4 h-rows = 1024B fp16
NR = T * D * WC * H              # 26624 table rows (< 32767, int16 idx)
OFF_I = D * WC * H               # t-offset step in rows  (3328)
OFF_J = WC * H                   # d-offset step in rows  (208)
F32 = mybir.dt.float32
F16 = mybir.dt.float16
I32 = mybir.dt.int32
I16 = mybir.dt.int16

_CACHED_NC = None


def _cubic_weights(nc, pool, f, nt):
    """DVE ops computing the 4 cubic B-spline weights of fractional position
    tile `f` ([P, nt] f32).  Returns 4 tiles [P, nt]."""
    v = nc.vector
    A = mybir.AluOpType
    f2 = pool.tile([P, nt], F32, tag="f2")
    f3 = pool.tile([P, nt], F32, tag="f3")
    v.tensor_tensor(out=f2[:], in0=f[:], in1=f[:], op=A.mult)
    v.tensor_tensor(out=f3[:], in0=f2[:], in1=f[:], op=A.mult)
    w0 = pool.tile([P, nt], F32, tag="w0")
    w1 = pool.tile([P, nt], F32, tag="w1")
    w2 = pool.tile([P, nt], F32, tag="w2")
    w3 = pool.tile([P, nt], F32, tag="w3")
    tmp = pool.tile([P, nt], F32, tag="wtmp")
    # w0 = (1-f)^3/6 = -(f-1)^3/6
    v.tensor_scalar(out=tmp[:], in0=f[:], scalar1=1.0, scalar2=None, op0=A.subtract)
    v.tensor_tensor(out=w0[:], in0=tmp[:], in1=tmp[:], op=A.mult)
    v.tensor_tensor(out=w0[:], in0=w0[:], in1=tmp[:], op=A.mult)
    v.tensor_scalar(out=w0[:], in0=w0[:], scalar1=-1.0 / 6.0, scalar2=None, op0=A.mult)
    # w1 = 2/3 - f2 + f3/2  ->  (f3*0.5 - f2) + 2/3
    v.scalar_tensor_tensor(out=w1[:], in0=f3[:], scalar=0.5, in1=f2[:],
                           op0=A.mult, op1=A.subtract)
    v.tensor_scalar(out=w1[:], in0=w1[:], scalar1=2.0 / 3.0, scalar2=None, op0=A.add)
    # w2 = 1/6 + (f + f2 - f3)/2
    v.tensor_tensor(out=w2[:], in0=f[:], in1=f2[:], op=A.add)
    v.tensor_tensor(out=w2[:], in0=w2[:], in1=f3[:], op=A.subtract)
    v.tensor_scalar(out=w2[:], in0=w2[:], scalar1=0.5, scalar2=1.0 / 6.0,
                    op0=A.mult, op1=A.add)
    # w3 = f3/6
    v.tensor_scalar(out=w3[:], in0=f3[:], scalar1=1.0 / 6.0, scalar2=None, op0=A.mult)
    return w0, w1, w2, w3


def _dim_prep(nc, pp, sp, U, dim, tag):
    """Window start O ([P,NT] f32, in 0..n-4) and boundary-transformed weights
    WT ([P,NT,4] f32) for one dim.  Identical math to the validated baseline:
    virtual linear-extrapolation padding folded into the end weights."""
    v = nc.vector
    A = mybir.AluOpType
    n = float(SIZES[dim])
    s = sp.tile([P, NT], F32, tag="s")
    v.tensor_scalar(out=s[:], in0=U[:, :, dim], scalar1=n - 1.0,
                    scalar2=None, op0=A.mult)
    # floor(s) via int cast round-trip; i = r - (s < r) is correct whether the
    # f32->i32 cast truncates or rounds-to-nearest (s >= 0 here).
    ri = sp.tile([P, NT], I32, tag="ri")
    v.tensor_copy(out=ri[:], in_=s[:])
    rf = sp.tile([P, NT], F32, tag="rf")
    v.tensor_copy(out=rf[:], in_=ri[:])
    flt = sp.tile([P, NT], F32, tag="flt")
    v.tensor_tensor(out=flt[:], in0=s[:], in1=rf[:], op=A.is_lt)
    ifl = sp.tile([P, NT], F32, tag="ifl")
    v.tensor_tensor(out=ifl[:], in0=rf[:], in1=flt[:], op=A.subtract)
    ic = sp.tile([P, NT], F32, tag="ic")
    v.tensor_scalar(out=ic[:], in0=ifl[:], scalar1=n - 2.0, scalar2=None, op0=A.min)
    f = sp.tile([P, NT], F32, tag="f")
    v.tensor_tensor(out=f[:], in0=s[:], in1=ic[:], op=A.subtract)
    mL = sp.tile([P, NT], F32, tag="mL")
    v.tensor_scalar(out=mL[:], in0=ic[:], scalar1=0.0, scalar2=None, op0=A.is_equal)
    mR = sp.tile([P, NT], F32, tag="mR")
    v.tensor_scalar(out=mR[:], in0=ic[:], scalar1=n - 2.0, scalar2=None,
                    op0=A.is_equal)
    # window start o = clip(i-1, 0, n-4)
    O = pp.tile([P, NT], F32, tag=f"O{tag}")
    v.tensor_scalar(out=O[:], in0=ic[:], scalar1=1.0, scalar2=0.0,
                    op0=A.subtract, op1=A.max)
    v.tensor_scalar(out=O[:], in0=O[:], scalar1=n - 4.0, scalar2=None, op0=A.min)

    w0, w1, w2, w3 = _cubic_weights(nc, sp, f, NT)
    # boundary delta vectors:
    #   left  (i==0):   wL = (w1+2w0, w2-w0, w3, 0)
    #   right (i==n-2): wR = (0, w0, w1-w3, w2+2w3)
    # w' = w + mL*(wL-w) + mR*(wR-w)
    WT = pp.tile([P, NT, 4], F32, tag=f"W{tag}")
    dl = sp.tile([P, NT], F32, tag="dl")
    dr = sp.tile([P, NT], F32, tag="dr")
    acc = sp.tile([P, NT], F32, tag="wacc")
    A_ = A
    # component 0: dL0 = w0+w1, dR0 = -w0
    v.tensor_tensor(out=dl[:], in0=w0[:], in1=w1[:], op=A_.add)
    v.tensor_tensor(out=dl[:], in0=dl[:], in1=mL[:], op=A_.mult)
    v.tensor_tensor(out=dr[:], in0=w0[:], in1=mR[:], op=A_.mult)
    v.tensor_tensor(out=acc[:], in0=w0[:], in1=dl[:], op=A_.add)
    v.tensor_tensor(out=WT[:, :, 0], in0=acc[:], in1=dr[:], op=A_.subtract)
    # component 1: dL1 = w2-w0-w1, dR1 = w0-w1
    v.tensor_tensor(out=dl[:], in0=w2[:], in1=w0[:], op=A_.subtract)
    v.tensor_tensor(out=dl[:], in0=dl[:], in1=w1[:], op=A_.subtract)
    v.tensor_tensor(out=dl[:], in0=dl[:], in1=mL[:], op=A_.mult)
    v.tensor_tensor(out=dr[:], in0=w0[:], in1=w1[:], op=A_.subtract)
    v.tensor_tensor(out=dr[:], in0=dr[:], in1=mR[:], op=A_.mult)
    v.tensor_tensor(out=acc[:], in0=w1[:], in1=dl[:], op=A_.add)
    v.tensor_tensor(out=WT[:, :, 1], in0=acc[:], in1=dr[:], op=A_.add)
    # component 2: dL2 = w3-w2, dR2 = w1-w2-w3
    v.tensor_tensor(out=dl[:], in0=w3[:], in1=w2[:], op=A_.subtract)
    v.tensor_tensor(out=dl[:], in0=dl[:], in1=mL[:], op=A_.mult)
    v.tensor_tensor(out=dr[:], in0=w1[:], in1=w2[:], op=A_.subtract)
    v.tensor_tensor(out=dr[:], in0=dr[:], in1=w3[:], op=A_.subtract)
    v.tensor_tensor(out=dr[:], in0=dr[:], in1=mR[:], op=A_.mult)
    v.tensor_tensor(out=acc[:], in0=w2[:], in1=dl[:], op=A_.add)
    v.tensor_tensor(out=WT[:, :, 2], in0=acc[:], in1=dr[:], op=A_.add)
    # component 3: dL3 = -w3, dR3 = w2+w3
    v.tensor_tensor(out=dl[:], in0=w3[:], in1=mL[:], op=A_.mult)
    v.tensor_tensor(out=dr[:], in0=w2[:], in1=w3[:], op=A_.add)
    v.tensor_tensor(out=dr[:], in0=dr[:], in1=mR[:], op=A_.mult)
    v.tensor_tensor(out=acc[:], in0=w3[:], in1=dl[:], op=A_.subtract)
    v.tensor_tensor(out=WT[:, :, 3], in0=acc[:], in1=dr[:], op=A_.add)
    return O, WT


def _build_nc():
    nc = bacc.Bacc("TRN2", target_bir_lowering=False, debug=False,
                   num_devices=NCORES, num_swdge_queues=2)
    u_in = nc.dram_tensor("u", [BSH, 4], F32, kind="ExternalInput")
    g_in = nc.dram_tensor("grid", [T * D * H * W, C], F32, kind="ExternalInput")
    out = nc.dram_tensor("out", [BSH, C], F32, kind="ExternalOutput")
    utab = nc.dram_tensor("utab", [NR, ROW], F16)
    bscr = nc.dram_tensor("bscr", [BSH], I16)

    v = nc.vector
    A = mybir.AluOpType

    with tile.TileContext(nc) as tc:
        with (
            tc.tile_pool(name="persist", bufs=1) as pp,
            tc.tile_pool(name="scratch", bufs=2) as sp,
            tc.tile_pool(name="gather", bufs=2) as gp,
            tc.tile_pool(name="stat", bufs=2) as stp,
            tc.tile_pool(name="red", bufs=2) as rp,
            tc.tile_pool(name="psv", bufs=2, space="PSUM") as psvp,
            tc.tile_pool(name="psm", bufs=2, space="PSUM") as psp,
        ):
            # ---------------- fp16 gather table build -----------------------
            # grid loaded in two h-halves to cap SBUF; f32->f16 conversion is
            # fused into the strided wc-window copies.
            with tc.tile_pool(name="build", bufs=1) as bp:
                G2 = bp.tile([P, WC, H, ROW], F16)    # [p=(t,d), wc, h, (4w,32c)]
                gview = g_in[:].rearrange("(p x) c -> p (x c)", p=P)
                HH = H // 2
                for half in range(2):
                    GF = bp.tile([P, HH * W * C], F32, tag="gf")
                    nc.sync.dma_start(
                        out=GF[:],
                        in_=gview[:, half * HH * W * C : (half + 1) * HH * W * C])
                    GFv = GF[:].rearrange("p (h w c) -> p h w c", h=HH, w=W)
                    for wc in range(WC):
                        v.tensor_copy(
                            out=G2[:, wc, half * HH : (half + 1) * HH, :],
                            in_=GFv[:, :, wc : wc + 4, :].rearrange(
                                "p h w c -> p h (w c)"))
                nc.sync.dma_start(
                    out=utab[:].rearrange("(p r) e -> p (r e)", p=P),
                    in_=G2[:].rearrange("p wc h e -> p (wc h e)"))

            # ---------------- per-query prep (q-major layout) ---------------
            U = pp.tile([P, NT, 4], F32)
            # query q = t*128 + p  ->  partition p, slot t
            nc.sync.dma_start(
                out=U[:], in_=u_in[:].rearrange("(t p) d -> p t d", p=P))

            Ot, WTt = _dim_prep(nc, pp, sp, U, 0, "t")
            Od, WTd = _dim_prep(nc, pp, sp, U, 1, "d")
            Oh, WTh = _dim_prep(nc, pp, sp, U, 2, "h")
            Ow, WTw = _dim_prep(nc, pp, sp, U, 3, "w")

            # base row index = ((ot*16 + od)*13 + ow)*16 + oh   (max 16012)
            base_f = pp.tile([P, NT], F32)
            v.scalar_tensor_tensor(out=base_f[:], in0=Ot[:], scalar=float(D),
                                   in1=Od[:], op0=A.mult, op1=A.add)
            v.scalar_tensor_tensor(out=base_f[:], in0=base_f[:], scalar=float(WC),
                                   in1=Ow[:], op0=A.mult, op1=A.add)
            v.scalar_tensor_tensor(out=base_f[:], in0=base_f[:], scalar=float(H),
                                   in1=Oh[:], op0=A.mult, op1=A.add)
            base_i = pp.tile([P, NT], I16)
            v.tensor_copy(out=base_i[:], in_=base_f[:])
            # bounce to DRAM in query order, then load replicated on all
            # partitions (the gather idx list wants bases along the free dim)
            nc.sync.dma_start(
                out=bscr[:].rearrange("(t p) -> p t", p=P), in_=base_i[:])

            # iota helpers (i32)
            pcol = pp.tile([P, 1], I32)
            nc.gpsimd.iota(out=pcol[:], pattern=[[0, 1]], base=0,
                           channel_multiplier=1)
            mrow = pp.tile([P, P], I32)
            nc.gpsimd.iota(out=mrow[:], pattern=[[1, P]], base=0,
                           channel_multiplier=0)
            grow = pp.tile([P, 16], I32)
            nc.gpsimd.iota(out=grow[:], pattern=[[1, 16]], base=0,
                           channel_multiplier=0)

            # per-partition gather-window offset: s = p%16 = (i,j), C-order;
            # offs = (s//4)*OFF_I + (s%4)*OFF_J
            scol = sp.tile([P, 1], I32, tag="scol")
            v.tensor_scalar(out=scol[:], in0=pcol[:], scalar1=15, scalar2=None,
                            op0=A.bitwise_and)
            icol = sp.tile([P, 1], I32, tag="icol")
            v.tensor_scalar(out=icol[:], in0=scol[:], scalar1=2, scalar2=None,
                            op0=A.arith_shift_right)
            jcol = sp.tile([P, 1], I32, tag="jcol")
            v.tensor_scalar(out=jcol[:], in0=scol[:], scalar1=3, scalar2=None,
                            op0=A.bitwise_and)
            offs32 = sp.tile([P, 1], I32, tag="offs32")
            v.tensor_scalar(out=offs32[:], in0=icol[:], scalar1=OFF_I,
                            scalar2=None, op0=A.mult)
            v.scalar_tensor_tensor(out=offs32[:], in0=jcol[:], scalar=OFF_J,
                                   in1=offs32[:], op0=A.mult, op1=A.add)
            offs16 = pp.tile([P, 1], I16)
            v.tensor_copy(out=offs16[:], in_=offs32[:])

            # full-shard gather index list: IDX[p, q] = base[q] + offs[p%16].
            # list pos n of tile t (n = g*128 + qhat*16 + s) reads idx
            # [n%16, n//16] = [s, q_local] -> unit lands on partition
            # n%128 = (qhat,s), slot n//128 = g.
            IDX = pp.tile([P, BSH], I16)
            nc.sync.dma_start(out=IDX[:], in_=bass.AP(bscr, 0, [[0, P], [1, BSH]]))
            v.tensor_tensor(out=IDX[:], in0=IDX[:],
                            in1=offs16[:, 0:1].to_broadcast([P, BSH]), op=A.add)

            # constant masks for the stationary builds (is_equal -> f16/f32
            # directly; only one transient i32 tile is ever live)
            X0 = pp.tile([P, 16], F16)
            MASK2 = pp.tile([P, P], F16)
            MASK = pp.tile([P, 16, P], F16)
            with tc.tile_pool(name="masks", bufs=1) as mp:
                # X0[q, g] = (q//8 == g)      (v-matmul moving operand)
                phi3 = mp.tile([P, 1], I32, tag="phi3")
                v.tensor_scalar(out=phi3[:], in0=pcol[:], scalar1=3,
                                scalar2=None, op0=A.arith_shift_right)
                v.tensor_tensor(out=X0[:], in0=grow[:],
                                in1=phi3[:, 0:1].to_broadcast([P, 16]),
                                op=A.is_equal)
                # MASK2[q, m] = (m//16 == q%8)    (St placement mask)
                mhi = mp.tile([P, P], I32, tag="mhi")
                v.tensor_scalar(out=mhi[:], in0=mrow[:], scalar1=4,
                                scalar2=None, op0=A.arith_shift_right)
                plo3 = mp.tile([P, 1], I32, tag="plo3")
                v.tensor_scalar(out=plo3[:], in0=pcol[:], scalar1=7,
                                scalar2=None, op0=A.bitwise_and)
                v.tensor_tensor(out=MASK2[:], in0=mhi[:],
                                in1=plo3[:, 0:1].to_broadcast([P, P]),
                                op=A.is_equal)
                # MASK[p=(qhat,s), g, m] = (m == 8g + p//16)  (block-diag mask)
                phi4 = mp.tile([P, 1], I32, tag="phi4")
                v.tensor_scalar(out=phi4[:], in0=pcol[:], scalar1=4,
                                scalar2=None, op0=A.arith_shift_right)
                g8 = mp.tile([P, 16], I32, tag="g8")
                v.tensor_scalar(out=g8[:], in0=grow[:], scalar1=8,
                                scalar2=None, op0=A.mult)
                mg = mp.tile([P, 16, P], I32, tag="mg")
                v.tensor_tensor(
                    out=mg[:],
                    in0=mrow[:].rearrange("p (x m) -> p x m", x=1).to_broadcast(
                        [P, 16, P]),
                    in1=g8[:].rearrange("p (g x) -> p g x", x=1).to_broadcast([P, 16, P]),
                    op=A.subtract)
                v.tensor_tensor(
                    out=MASK[:], in0=mg[:],
                    in1=phi4[:, 0:1].rearrange("p (x y) -> p x y", x=1).to_broadcast(
                        [P, 16, P]),
                    op=A.is_equal)

            # ---------------- per-tile gather + PE reduce -------------------
            # rows NR-3..NR-1 are only ever read as the tail of an element
            # starting at <= NR-4, so the formal AP row count is NR-3 to stay
            # in bounds: (NR-4)*ROW + UNIT == NR*ROW exactly.
            utab_ap = bass.AP(utab, 0, [[ROW, NR - 3], [1, UNIT]])
            for t in range(NT):
                X = gp.tile([P, 16, UNIT], F16, tag="X")
                for k in range(2):
                    nc.gpsimd.dma_gather(
                        out_ap=X[:, 8 * k : 8 * (k + 1), :],
                        in_ap=utab_ap,
                        idxs_ap=IDX[:, t * P + 64 * k : t * P + 64 * k + 64],
                        num_idxs=P * 8,
                        num_idxs_reg=P * 8,
                        elem_size=UNIT,
                        elem_step=ROW,
                        queue_num=k,
                    )

                # per-query (t,d) weight products, q-major: wtd[q, s=(i,j)]
                wtd = stp.tile([P, 4, 4], F16, tag="wtd")
                v.tensor_tensor(
                    out=wtd[:],
                    in0=WTt[:, t, :].rearrange("p (i x) -> p i x", x=1).to_broadcast(
                        [P, 4, 4]),
                    in1=WTd[:, t, :].rearrange("p (x j) -> p x j", x=1).to_broadcast(
                        [P, 4, 4]),
                    op=A.mult)
                # St[q, 16*(q%8)+s] = wtd[q, s]; PE broadcast-matmul turns this
                # into v[(qhat,s), g] = wtd[8g+qhat, s] without a DMA bounce.
                St = stp.tile([P, P], F16, tag="St")
                v.tensor_tensor(
                    out=St[:].rearrange("p (r s) -> p r s", r=8),
                    in0=wtd[:].rearrange("p i j -> p (i j)").rearrange("p (x s) -> p x s", x=1).to_broadcast(
                        [P, 8, 16]),
                    in1=MASK2[:].rearrange("p (r s) -> p r s", r=8),
                    op=A.mult)
                PSV = psvp.tile([P, 16], F32, tag="psv")
                nc.tensor.matmul(PSV[:], St[:], X0[:], start=True, stop=True)
                vsb = stp.tile([P, 16], F16, tag="vsb")
                v.tensor_copy(out=vsb[:], in_=PSV[:])
                # block-diagonal stationaries for the 16 accumulating matmuls
                SA = stp.tile([P, 16, P], F16, tag="SA")
                v.tensor_tensor(
                    out=SA[:],
                    in0=vsb[:].rearrange("p (g x) -> p g x", x=1).to_broadcast([P, 16, P]),
                    in1=MASK[:],
                    op=A.mult)

                PS = psp.tile([P, UNIT], F32, tag="ps")
                for g in range(16):
                    nc.tensor.matmul(PS[:], SA[:, g, :], X[:, g, :],
                                     start=(g == 0), stop=(g == 15))

                # PS[q, (h,w,c)] -> weighted h then w reduction on DVE
                PSh = PS[:].rearrange("p (h e) -> p h e", h=4)
                a0 = rp.tile([P, 4 * C], F32, tag="a0")
                a1 = rp.tile([P, 4 * C], F32, tag="a1")
                v.tensor_scalar(out=a0[:], in0=PSh[:, 0, :],
                                scalar1=WTh[:, t, 0:1], scalar2=None, op0=A.mult)
                v.scalar_tensor_tensor(out=a1[:], in0=PSh[:, 1, :],
                                       scalar=WTh[:, t, 1:2], in1=a0[:],
                                       op0=A.mult, op1=A.add)
                v.scalar_tensor_tensor(out=a0[:], in0=PSh[:, 2, :],
                                       scalar=WTh[:, t, 2:3], in1=a1[:],
                                       op0=A.mult, op1=A.add)
                v.scalar_tensor_tensor(out=a1[:], in0=PSh[:, 3, :],
                                       scalar=WTh[:, t, 3:4], in1=a0[:],
                                       op0=A.mult, op1=A.add)
                aw = a1[:].rearrange("p (w c) -> p w c", w=4)
                o0 = rp.tile([P, C], F32, tag="o0")
                o1 = rp.tile([P, C], F32, tag="o1")
                v.tensor_scalar(out=o0[:], in0=aw[:, 0, :],
                                scalar1=WTw[:, t, 0:1], scalar2=None, op0=A.mult)
                v.scalar_tensor_tensor(out=o1[:], in0=aw[:, 1, :],
                                       scalar=WTw[:, t, 1:2], in1=o0[:],
                                       op0=A.mult, op1=A.add)
                v.scalar_tensor_tensor(out=o0[:], in0=aw[:, 2, :],
                                       scalar=WTw[:, t, 2:3], in1=o1[:],
                                       op0=A.mult, op1=A.add)
                o2 = rp.tile([P, C], F32, tag="o2")
                v.scalar_tensor_tensor(out=o2[:], in0=aw[:, 3, :],
                                       scalar=WTw[:, t, 3:4], in1=o0[:],
                                       op0=A.mult, op1=A.add)
                nc.sync.dma_start(out=out[t * P : (t + 1) * P, :], in_=o2[:])

    nc.compile()
    return nc


def _get_nc():
    global _CACHED_NC
    if _CACHED_NC is None:
        _CACHED_NC = _build_nc()
    return _CACHED_NC


def kernel(u: np.ndarray, grid: np.ndarray) -> np.ndarray:
    u = np.ascontiguousarray(np.asarray(u, dtype=np.float32))
    grid = np.ascontiguousarray(np.asarray(grid, dtype=np.float32))
    gflat = grid.reshape(-1, C)
    nc = _get_nc()
    in_maps = [
        {"u": u[c * BSH : (c + 1) * BSH], "grid": gflat}
        for c in range(NCORES)
    ]
    res = run_bass_kernel_spmd(nc, in_maps, list(range(NCORES)))
    return np.concatenate([res.results[c]["out"] for c in range(NCORES)], axis=0)


if __name__ == "__main__":
    rng = np.random.default_rng(0)
    u = rng.random((B, 4), dtype=np.float32)
    grid = rng.standard_normal((*SIZES, C), dtype=np.float32)
    out = kernel(u, grid)
    print(out.shape, out.dtype)


# revision 16
# speedup vs baseline: 2.0422x; 1.1100x over previous
"""Trainium2 Bass kernel for 4D cubic B-spline grid evaluation.

Problem: for each of 65536 query coords u in [0,1)^4, evaluate a uniform cubic
B-spline over an (8,16,16,16) control grid with 32 channels and linear-
extrapolation padding -> output (65536, 32) f32.

Strategy (data-parallel over the query batch, 8 cores x 8192 queries):
  * Linear-extrapolation padding is folded into transformed boundary weights
    (no padded grid is materialized).
  * The grid is re-laid once in DRAM as an fp16 table with 256B rows
    [t, d, wc, h] -> [4 w-points x 32 ch].  A gather element is 1024B spanning
    4 consecutive h-rows (elem_step=128 els < elem_size=512 els), so one
    descriptor fetches a full [4h, 4w, 32c] window pencil at >=512B, i.e. at
    full DMA-bus rate while fp16 halves the gathered bytes (16KB/query).
  * Per query only 4x4 (t,d) descriptors remain: 16 units land on partitions
    p=(qhat, s) (8 queries x 16 (t,d)-slots per 128-partition group).
  * The 16-slot (t,d)-contraction runs on the TensorEngine: 16 accumulating
    matmuls per 128-query tile with block-diagonal stationaries carrying the
    per-query wt*wd weights; PSUM ends up [128 q, 4h*4w*32c].
  * Per-query weight layout transforms (q-major -> (qhat,s)-major) are done by
    a tiny PE broadcast-matmul instead of DMA bounces.
  * DVE finishes with per-partition-scalar FMA chains over h then w.
"""

import numpy as np

import concourse.bacc as bacc
import concourse.bass as bass
import concourse.mybir as mybir
import concourse.tile as tile
from concourse.bass_utils import run_bass_kernel_spmd

P = 128              # partitions / queries per tile
NT = 64              # tiles per core
BSH = P * NT         # 8192 queries per core
NCORES = 8
B = BSH * NCORES     # 65536
C = 32
SIZES = (8, 16, 16, 16)          # t, d, h, w control-point resolution
T, D, H, W = SIZES
WC = 13                          # distinct clamped w-window starts (0..12)
WCL = 4                          # w-window starts shipped per core (host bins)
WS = WCL + 3                     # w-points per core's grid slice (7)
ROW = 4 * C                      # table row: 4 w-points x 32 ch fp16 = 256B
UNIT = 4 * ROW                   # gather element: 4 h-rows = 1024B fp16
NR = T * D * WCL * H             # 8192 table rows per core (int16 idx)
# wc-major row layout [wc, t, d, h]: chunk per wc is contiguous, so the table
# build pipelines 13 chunk writes with the on-chip window copies.
OFF_I = D * H                    # t-offset step in rows  (256)
OFF_J = H                        # d-offset step in rows  (16)
F32 = mybir.dt.float32
F16 = mybir.dt.float16
I32 = mybir.dt.int32
I16 = mybir.dt.int16

_CACHED_NC = None


def _cubic_weights(nc, pool, f, nt):
    """DVE ops computing the 4 cubic B-spline weights of fractional position
    tile `f` ([P, nt] f32).  Returns 4 tiles [P, nt]."""
    v = nc.vector
    A = mybir.AluOpType
    f2 = pool.tile([P, nt], F32, tag="f2")
    f3 = pool.tile([P, nt], F32, tag="f3")
    v.tensor_tensor(out=f2[:], in0=f[:], in1=f[:], op=A.mult)
    v.tensor_tensor(out=f3[:], in0=f2[:], in1=f[:], op=A.mult)
    w0 = pool.tile([P, nt], F32, tag="w0")
    w1 = pool.tile([P, nt], F32, tag="w1")
    w2 = pool.tile([P, nt], F32, tag="w2")
    w3 = pool.tile([P, nt], F32, tag="w3")
    tmp = pool.tile([P, nt], F32, tag="wtmp")
    # w0 = (1-f)^3/6 = -(f-1)^3/6
    v.tensor_scalar(out=tmp[:], in0=f[:], scalar1=1.0, scalar2=None, op0=A.subtract)
    v.tensor_tensor(out=w0[:], in0=tmp[:], in1=tmp[:], op=A.mult)
    v.tensor_tensor(out=w0[:], in0=w0[:], in1=tmp[:], op=A.mult)
    v.tensor_scalar(out=w0[:], in0=w0[:], scalar1=-1.0 / 6.0, scalar2=None, op0=A.mult)
    # w1 = 2/3 - f2 + f3/2  ->  (f3*0.5 - f2) + 2/3
    v.scalar_tensor_tensor(out=w1[:], in0=f3[:], scalar=0.5, in1=f2[:],
                           op0=A.mult, op1=A.subtract)
    v.tensor_scalar(out=w1[:], in0=w1[:], scalar1=2.0 / 3.0, scalar2=None, op0=A.add)
    # w2 = 1/6 + (f + f2 - f3)/2
    v.tensor_tensor(out=w2[:], in0=f[:], in1=f2[:], op=A.add)
    v.tensor_tensor(out=w2[:], in0=w2[:], in1=f3[:], op=A.subtract)
    v.tensor_scalar(out=w2[:], in0=w2[:], scalar1=0.5, scalar2=1.0 / 6.0,
                    op0=A.mult, op1=A.add)
    # w3 = f3/6
    v.tensor_scalar(out=w3[:], in0=f3[:], scalar1=1.0 / 6.0, scalar2=None, op0=A.mult)
    return w0, w1, w2, w3


def _dim_offsets(nc, pp, sp, U, dim, tag):
    """Clamped cell index ic, fractional f and window start O for one dim.
    Emitted for all dims FIRST so the gather index chain (base -> bscr ->
    IDX) clears DVE before the weight chains; weights then overlap the
    first gather transfers."""
    v = nc.vector
    A = mybir.AluOpType
    n = float(SIZES[dim])
    s = sp.tile([P, NT], F32, tag="s")
    v.tensor_scalar(out=s[:], in0=U[:, :, dim], scalar1=n - 1.0,
                    scalar2=None, op0=A.mult)
    # floor(s) via int cast round-trip; i = r - (s < r) is correct whether the
    # f32->i32 cast truncates or rounds-to-nearest (s >= 0 here).
    ri = sp.tile([P, NT], I32, tag="ri")
    v.tensor_copy(out=ri[:], in_=s[:])
    rf = sp.tile([P, NT], F32, tag="rf")
    v.tensor_copy(out=rf[:], in_=ri[:])
    flt = sp.tile([P, NT], F32, tag="flt")
    v.tensor_tensor(out=flt[:], in0=s[:], in1=rf[:], op=A.is_lt)
    ifl = sp.tile([P, NT], F32, tag="ifl")
    v.tensor_tensor(out=ifl[:], in0=rf[:], in1=flt[:], op=A.subtract)
    ic = pp.tile([P, NT], F32, tag=f"ic{tag}")
    v.tensor_scalar(out=ic[:], in0=ifl[:], scalar1=n - 2.0, scalar2=None, op0=A.min)
    f = pp.tile([P, NT], F32, tag=f"f{tag}")
    v.tensor_tensor(out=f[:], in0=s[:], in1=ic[:], op=A.subtract)
    # window start o = clip(i-1, 0, n-4)
    O = pp.tile([P, NT], F32, tag=f"O{tag}")
    v.tensor_scalar(out=O[:], in0=ic[:], scalar1=1.0, scalar2=0.0,
                    op0=A.subtract, op1=A.max)
    v.tensor_scalar(out=O[:], in0=O[:], scalar1=n - 4.0, scalar2=None, op0=A.min)
    return O, ic, f


def _dim_weights(nc, pp, sp, ic, f, dim, tag):
    """Boundary-transformed cubic weights WT ([P,NT,4] f32) for one dim."""
    v = nc.vector
    A = mybir.AluOpType
    n = float(SIZES[dim])
    mL = sp.tile([P, NT], F32, tag="mL")
    v.tensor_scalar(out=mL[:], in0=ic[:], scalar1=0.0, scalar2=None, op0=A.is_equal)
    mR = sp.tile([P, NT], F32, tag="mR")
    v.tensor_scalar(out=mR[:], in0=ic[:], scalar1=n - 2.0, scalar2=None,
                    op0=A.is_equal)
    w0, w1, w2, w3 = _cubic_weights(nc, sp, f, NT)
    # boundary delta vectors:
    #   left  (i==0):   wL = (w1+2w0, w2-w0, w3, 0)
    #   right (i==n-2): wR = (0, w0, w1-w3, w2+2w3)
    # w' = w + mL*(wL-w) + mR*(wR-w)
    WT = pp.tile([P, NT, 4], F32, tag=f"W{tag}")
    dl = sp.tile([P, NT], F32, tag="dl")
    dr = sp.tile([P, NT], F32, tag="dr")
    acc = sp.tile([P, NT], F32, tag="wacc")
    A_ = A
    # component 0: dL0 = w0+w1, dR0 = -w0
    v.tensor_tensor(out=dl[:], in0=w0[:], in1=w1[:], op=A_.add)
    v.tensor_tensor(out=dl[:], in0=dl[:], in1=mL[:], op=A_.mult)
    v.tensor_tensor(out=dr[:], in0=w0[:], in1=mR[:], op=A_.mult)
    v.tensor_tensor(out=acc[:], in0=w0[:], in1=dl[:], op=A_.add)
    v.tensor_tensor(out=WT[:, :, 0], in0=acc[:], in1=dr[:], op=A_.subtract)
    # component 1: dL1 = w2-w0-w1, dR1 = w0-w1
    v.tensor_tensor(out=dl[:], in0=w2[:], in1=w0[:], op=A_.subtract)
    v.tensor_tensor(out=dl[:], in0=dl[:], in1=w1[:], op=A_.subtract)
    v.tensor_tensor(out=dl[:], in0=dl[:], in1=mL[:], op=A_.mult)
    v.tensor_tensor(out=dr[:], in0=w0[:], in1=w1[:], op=A_.subtract)
    v.tensor_tensor(out=dr[:], in0=dr[:], in1=mR[:], op=A_.mult)
    v.tensor_tensor(out=acc[:], in0=w1[:], in1=dl[:], op=A_.add)
    v.tensor_tensor(out=WT[:, :, 1], in0=acc[:], in1=dr[:], op=A_.add)
    # component 2: dL2 = w3-w2, dR2 = w1-w2-w3
    v.tensor_tensor(out=dl[:], in0=w3[:], in1=w2[:], op=A_.subtract)
    v.tensor_tensor(out=dl[:], in0=dl[:], in1=mL[:], op=A_.mult)
    v.tensor_tensor(out=dr[:], in0=w1[:], in1=w2[:], op=A_.subtract)
    v.tensor_tensor(out=dr[:], in0=dr[:], in1=w3[:], op=A_.subtract)
    v.tensor_tensor(out=dr[:], in0=dr[:], in1=mR[:], op=A_.mult)
    v.tensor_tensor(out=acc[:], in0=w2[:], in1=dl[:], op=A_.add)
    v.tensor_tensor(out=WT[:, :, 2], in0=acc[:], in1=dr[:], op=A_.add)
    # component 3: dL3 = -w3, dR3 = w2+w3
    v.tensor_tensor(out=dl[:], in0=w3[:], in1=mL[:], op=A_.mult)
    v.tensor_tensor(out=dr[:], in0=w2[:], in1=w3[:], op=A_.add)
    v.tensor_tensor(out=dr[:], in0=dr[:], in1=mR[:], op=A_.mult)
    v.tensor_tensor(out=acc[:], in0=w3[:], in1=dl[:], op=A_.subtract)
    v.tensor_tensor(out=WT[:, :, 3], in0=acc[:], in1=dr[:], op=A_.add)
    return WT


def _build_nc(no_reduce=False, no_gather=False):
    nc = bacc.Bacc("TRN2", target_bir_lowering=False, debug=False,
                   num_devices=NCORES, num_swdge_queues=2)
    u_in = nc.dram_tensor("u", [BSH, 4], F32, kind="ExternalInput")
    g_in = nc.dram_tensor("grid", [T * D * H * WS, C], F32,
                          kind="ExternalInput")
    wofs = nc.dram_tensor("wofs", [1], F32, kind="ExternalInput")
    out = nc.dram_tensor("out", [BSH, C], F16, kind="ExternalOutput")
    utab = nc.dram_tensor("utab", [NR, ROW], F16)
    bscr = nc.dram_tensor("bscr", [BSH], I16)

    v = nc.vector
    A = mybir.AluOpType

    with tile.TileContext(nc) as tc:
        with (
            tc.tile_pool(name="persist", bufs=1) as pp,
            tc.tile_pool(name="scratch", bufs=2) as sp,
            tc.tile_pool(name="gather", bufs=3) as gp,
            tc.tile_pool(name="stat", bufs=2) as stp,
            tc.tile_pool(name="red", bufs=2) as rp,
            tc.tile_pool(name="psv", bufs=2, space="PSUM") as psvp,
            tc.tile_pool(name="psm", bufs=2, space="PSUM") as psp,
        ):
            # query q = p*64 + j  ->  partition p, slot j (contiguous u rows
            # per partition: 128 big DMA descriptors instead of 8192 tiny)
            U = pp.tile([P, NT, 4], F32)
            nc.sync.dma_start(
                out=U[:], in_=u_in[:].rearrange("(p x) d -> p x d", p=P))

            # ---------------- fp16 gather table build -----------------------
            # f32->f16 conversion on the (otherwise idle) Activation engine;
            # per-wc window copies on DVE at 4x; each contiguous wc-chunk is
            # written to DRAM as soon as its copy lands (pipelined with the
            # remaining copies and the Phase-A DMAs).
            with tc.tile_pool(name="build", bufs=1) as bp:
                GF32 = bp.tile([P, H * WS * C], F32)   # [p=(t,d), (h,7w,c)]
                nc.sync.dma_start(
                    out=GF32[:],
                    in_=g_in[:].rearrange("(p x) c -> p (x c)", p=P))
                GF16 = bp.tile([P, H * WS * C], F16)
                nc.scalar.activation(
                    out=GF16[:], in_=GF32[:],
                    func=mybir.ActivationFunctionType.Copy)
                GFv = GF16[:].rearrange("p (h w c) -> p h w c", h=H, w=WS)
                with tc.tile_pool(name="buildc", bufs=4) as bcp:
                    for wc in range(WCL):
                        G2c = bcp.tile([P, H, ROW], F16, tag="g2c")
                        v.tensor_copy(
                            out=G2c[:],
                            in_=GFv[:, :, wc : wc + 4, :].rearrange(
                                "p h w c -> p h (w c)"))
                        nc.sync.dma_start(
                            out=utab[wc * T * D * H : (wc + 1) * T * D * H,
                                     :].rearrange("(p h) e -> p (h e)", p=P),
                            in_=G2c[:].rearrange("p h e -> p (h e)"))

            # ---------------- per-query prep ---------------------------------
            Ot, ict, ft = _dim_offsets(nc, pp, sp, U, 0, "t")
            Od, icd, fd = _dim_offsets(nc, pp, sp, U, 1, "d")
            Oh, ich, fh = _dim_offsets(nc, pp, sp, U, 2, "h")
            Ow, icw, fw = _dim_offsets(nc, pp, sp, U, 3, "w")

            # per-core w-window offset (host bins queries by ow)
            WOF = pp.tile([P, 1], F32)
            nc.sync.dma_start(out=WOF[:], in_=bass.AP(wofs, 0, [[0, P], [1, 1]]))
            Owl = pp.tile([P, NT], F32)
            v.tensor_tensor(out=Owl[:], in0=Ow[:],
                            in1=WOF[:, 0:1].to_broadcast([P, NT]),
                            op=A.subtract)
            # base row index = ((ow_local*8 + ot)*16 + od)*16 + oh  (max 8188)
            base_f = pp.tile([P, NT], F32)
            v.scalar_tensor_tensor(out=base_f[:], in0=Owl[:], scalar=float(T),
                                   in1=Ot[:], op0=A.mult, op1=A.add)
            v.scalar_tensor_tensor(out=base_f[:], in0=base_f[:], scalar=float(D),
                                   in1=Od[:], op0=A.mult, op1=A.add)
            v.scalar_tensor_tensor(out=base_f[:], in0=base_f[:], scalar=float(H),
                                   in1=Oh[:], op0=A.mult, op1=A.add)
            base_i = pp.tile([P, NT], I16)
            v.tensor_copy(out=base_i[:], in_=base_f[:])
            # bounce to DRAM in q-order (contiguous per partition, 128 big
            # descriptors); the per-tile index build strides across it
            nc.sync.dma_start(
                out=bscr[:].rearrange("(p j) -> p j", p=P), in_=base_i[:])

            # iota helpers (i32)
            pcol = pp.tile([P, 1], I32)
            nc.gpsimd.iota(out=pcol[:], pattern=[[0, 1]], base=0,
                           channel_multiplier=1)
            mrow = pp.tile([P, P], I32)
            nc.gpsimd.iota(out=mrow[:], pattern=[[1, P]], base=0,
                           channel_multiplier=0)
            grow = pp.tile([P, 16], I32)
            nc.gpsimd.iota(out=grow[:], pattern=[[1, 16]], base=0,
                           channel_multiplier=0)

            # per-partition gather-window offset: s = p%16 = (i,j), C-order;
            # offs = (s//4)*OFF_I + (s%4)*OFF_J
            scol = sp.tile([P, 1], I32, tag="scol")
            v.tensor_scalar(out=scol[:], in0=pcol[:], scalar1=15, scalar2=None,
                            op0=A.bitwise_and)
            icol = sp.tile([P, 1], I32, tag="icol")
            v.tensor_scalar(out=icol[:], in0=scol[:], scalar1=2, scalar2=None,
                            op0=A.arith_shift_right)
            jcol = sp.tile([P, 1], I32, tag="jcol")
            v.tensor_scalar(out=jcol[:], in0=scol[:], scalar1=3, scalar2=None,
                            op0=A.bitwise_and)
            offs32 = sp.tile([P, 1], I32, tag="offs32")
            v.tensor_scalar(out=offs32[:], in0=icol[:], scalar1=OFF_I,
                            scalar2=None, op0=A.mult)
            v.scalar_tensor_tensor(out=offs32[:], in0=jcol[:], scalar=OFF_J,
                                   in1=offs32[:], op0=A.mult, op1=A.add)
            offs16 = pp.tile([P, 1], I16)
            v.tensor_copy(out=offs16[:], in_=offs32[:])

            # full-shard gather index list: IDX[p, q] = base[q] + offs[p%16].
            # list pos n of tile t (n = g*128 + qhat*16 + s) reads idx
            # [n%16, n//16] = [s, q_local] -> unit lands on partition
            # n%128 = (qhat,s), slot n//128 = g.
            IDX = pp.tile([P, BSH], I16)
            nc.sync.dma_start(out=IDX[:], in_=bass.AP(bscr, 0, [[0, P], [1, BSH]]))

            # weights can compute while the first gathers stream
            WTt = _dim_weights(nc, pp, sp, ict, ft, 0, "t")
            WTd = _dim_weights(nc, pp, sp, icd, fd, 1, "d")
            WTh = _dim_weights(nc, pp, sp, ich, fh, 2, "h")
            WTw = _dim_weights(nc, pp, sp, icw, fw, 3, "w")

            # constant masks for the stationary builds (is_equal -> f16/f32
            # directly; only one transient i32 tile is ever live)
            X0 = pp.tile([P, 16], F16)
            MASK2 = pp.tile([P, P], F16)
            MASK = pp.tile([P, 16, P], F16)
            with tc.tile_pool(name="masks", bufs=1) as mp:
                # X0[q, g] = (q//8 == g)      (v-matmul moving operand)
                phi3 = mp.tile([P, 1], I32, tag="phi3")
                v.tensor_scalar(out=phi3[:], in0=pcol[:], scalar1=3,
                                scalar2=None, op0=A.arith_shift_right)
                v.tensor_tensor(out=X0[:], in0=grow[:],
                                in1=phi3[:, 0:1].to_broadcast([P, 16]),
                                op=A.is_equal)
                # MASK2[q, m] = (m//16 == q%8)    (St placement mask)
                mhi = mp.tile([P, P], I32, tag="mhi")
                v.tensor_scalar(out=mhi[:], in0=mrow[:], scalar1=4,
                                scalar2=None, op0=A.arith_shift_right)
                plo3 = mp.tile([P, 1], I32, tag="plo3")
                v.tensor_scalar(out=plo3[:], in0=pcol[:], scalar1=7,
                                scalar2=None, op0=A.bitwise_and)
                v.tensor_tensor(out=MASK2[:], in0=mhi[:],
                                in1=plo3[:, 0:1].to_broadcast([P, P]),
                                op=A.is_equal)
                # MASK[p=(qhat,s), g, m] = (m == 8g + p//16)  (block-diag mask)
                phi4 = mp.tile([P, 1], I32, tag="phi4")
                v.tensor_scalar(out=phi4[:], in0=pcol[:], scalar1=4,
                                scalar2=None, op0=A.arith_shift_right)
                g8 = mp.tile([P, 16], I32, tag="g8")
                v.tensor_scalar(out=g8[:], in0=grow[:], scalar1=8,
                                scalar2=None, op0=A.mult)
                mg = mp.tile([P, 16, P], I32, tag="mg")
                v.tensor_tensor(
                    out=mg[:],
                    in0=mrow[:].rearrange("p (x m) -> p x m", x=1).to_broadcast(
                        [P, 16, P]),
                    in1=g8[:].rearrange("p (g x) -> p g x", x=1).to_broadcast([P, 16, P]),
                    op=A.subtract)
                v.tensor_tensor(
                    out=MASK[:], in0=mg[:],
                    in1=phi4[:, 0:1].rearrange("p (x y) -> p x y", x=1).to_broadcast(
                        [P, 16, P]),
                    op=A.is_equal)

            # ---------------- per-tile gather + PE reduce -------------------
            # rows NR-3..NR-1 are only ever read as the tail of an element
            # starting at <= NR-4, so the formal AP row count is NR-3 to stay
            # in bounds: (NR-4)*ROW + UNIT == NR*ROW exactly.
            utab_ap = bass.AP(utab, 0, [[ROW, NR - 3], [1, UNIT]])
            for t in range(NT):
                X = gp.tile([P, 16, UNIT], F16, tag="X")
                IDXt = stp.tile([P, P], I16, tag="idxt")
                # IDXt[p', l] = base(q = 64*l + t) + offs[p' % 16]
                v.tensor_tensor(out=IDXt[:],
                                in0=IDX[:].rearrange("p (l j) -> p l j", j=NT)[
                                    :, :, t],
                                in1=offs16[:, 0:1].to_broadcast([P, P]),
                                op=A.add)
                for k in (range(0) if no_gather else range(2)):
                    nc.gpsimd.dma_gather(
                        out_ap=X[:, 8 * k : 8 * (k + 1), :],
                        in_ap=utab_ap,
                        idxs_ap=IDXt[:, 64 * k : 64 * k + 64],
                        num_idxs=P * 8,
                        num_idxs_reg=P * 8,
                        elem_size=UNIT,
                        elem_step=ROW,
                        queue_num=k,
                    )

                if no_reduce:
                    oz = rp.tile([P, C], F16, tag="oz")
                    v.tensor_copy(out=oz[:], in_=X[:, 0, 0:C])
                    nc.sync.dma_start(
                        out=out[:].rearrange("(p j) c -> p j c", p=P)[:, t, :],
                        in_=oz[:])
                    continue
                # per-query (t,d) weight products, q-major: wtd[q, s=(i,j)]
                wtd = stp.tile([P, 4, 4], F16, tag="wtd")
                v.tensor_tensor(
                    out=wtd[:],
                    in0=WTt[:, t, :].rearrange("p (i x) -> p i x", x=1).to_broadcast(
                        [P, 4, 4]),
                    in1=WTd[:, t, :].rearrange("p (x j) -> p x j", x=1).to_broadcast(
                        [P, 4, 4]),
                    op=A.mult)
                # St[q, 16*(q%8)+s] = wtd[q, s]; PE broadcast-matmul turns this
                # into v[(qhat,s), g] = wtd[8g+qhat, s] without a DMA bounce.
                St = stp.tile([P, P], F16, tag="St")
                v.tensor_tensor(
                    out=St[:].rearrange("p (r s) -> p r s", r=8),
                    in0=wtd[:].rearrange("p i j -> p (i j)").rearrange("p (x s) -> p x s", x=1).to_broadcast(
                        [P, 8, 16]),
                    in1=MASK2[:].rearrange("p (r s) -> p r s", r=8),
                    op=A.mult)
                PSV = psvp.tile([P, 16], F32, tag="psv")
                nc.tensor.matmul(PSV[:], St[:], X0[:], start=True, stop=True)
                vsb = stp.tile([P, 16], F16, tag="vsb")
                v.tensor_copy(out=vsb[:], in_=PSV[:])
                # block-diagonal stationaries for the 16 accumulating matmuls
                SA = stp.tile([P, 16, P], F16, tag="SA")
                v.tensor_tensor(
                    out=SA[:],
                    in0=vsb[:].rearrange("p (g x) -> p g x", x=1).to_broadcast([P, 16, P]),
                    in1=MASK[:],
                    op=A.mult)

                PS = psp.tile([P, UNIT], F32, tag="ps")
                if no_gather:
                    nc.tensor.matmul(PS[:, :P], SA[:, 0, :], MASK[:, 0, :],
                                     start=True, stop=True)
                else:
                    for g in range(16):
                        nc.tensor.matmul(PS[:], SA[:, g, :], X[:, g, :],
                                         start=(g == 0), stop=(g == 15))

                # PS[q, (h,w,c)] -> weighted h then w reduction on DVE
                PSh = PS[:].rearrange("p (h e) -> p h e", h=4)
                a0 = rp.tile([P, 4 * C], F32, tag="a0")
                a1 = rp.tile([P, 4 * C], F32, tag="a1")
                v.tensor_scalar(out=a0[:], in0=PSh[:, 0, :],
                                scalar1=WTh[:, t, 0:1], scalar2=None, op0=A.mult)
                v.scalar_tensor_tensor(out=a1[:], in0=PSh[:, 1, :],
                                       scalar=WTh[:, t, 1:2], in1=a0[:],
                                       op0=A.mult, op1=A.add)
                v.scalar_tensor_tensor(out=a0[:], in0=PSh[:, 2, :],
                                       scalar=WTh[:, t, 2:3], in1=a1[:],
                                       op0=A.mult, op1=A.add)
                v.scalar_tensor_tensor(out=a1[:], in0=PSh[:, 3, :],
                                       scalar=WTh[:, t, 3:4], in1=a0[:],
                                       op0=A.mult, op1=A.add)
                aw = a1[:].rearrange("p (w c) -> p w c", w=4)
                o0 = rp.tile([P, C], F32, tag="o0")
                o1 = rp.tile([P, C], F32, tag="o1")
                v.tensor_scalar(out=o0[:], in0=aw[:, 0, :],
                                scalar1=WTw[:, t, 0:1], scalar2=None, op0=A.mult)
                v.scalar_tensor_tensor(out=o1[:], in0=aw[:, 1, :],
                                       scalar=WTw[:, t, 1:2], in1=o0[:],
                                       op0=A.mult, op1=A.add)
                v.scalar_tensor_tensor(out=o0[:], in0=aw[:, 2, :],
                                       scalar=WTw[:, t, 2:3], in1=o1[:],
                                       op0=A.mult, op1=A.add)
                o2 = rp.tile([P, C], F16, tag="o2")
                v.scalar_tensor_tensor(out=o2[:], in0=aw[:, 3, :],
                                       scalar=WTw[:, t, 3:4], in1=o0[:],
                                       op0=A.mult, op1=A.add)
                nc.sync.dma_start(
                    out=out[:].rearrange("(p j) c -> p j c", p=P)[:, t, :],
                    in_=o2[:])

    nc.compile()
    return nc


def _get_nc():
    global _CACHED_NC
    if _CACHED_NC is None:
        _CACHED_NC = _build_nc()
    return _CACHED_NC


def kernel(u: np.ndarray, grid: np.ndarray) -> np.ndarray:
    u = np.ascontiguousarray(np.asarray(u, dtype=np.float32))
    grid = np.asarray(grid, dtype=np.float32).reshape(T, D, H, W, C)
    # Host-side binning: sort queries by their w-window start so each core
    # only needs a 7-wide w-slice of the grid (shrinks per-core table build).
    sw = u[:, 3] * (W - 1.0)
    isw = np.clip(np.floor(sw), 0.0, W - 2.0)
    ow = np.clip(isw - 1.0, 0.0, W - 4.0).astype(np.int64)   # 0..12
    order = np.argsort(ow, kind="stable")
    nc = _get_nc()
    in_maps = []
    for c in range(NCORES):
        qi = order[c * BSH : (c + 1) * BSH]
        owc = ow[qi]
        wc0 = int(min(owc.min(), W - 4 - 3))
        assert owc.max() - wc0 <= WCL - 1, "w-window span exceeds slice"
        gs = np.ascontiguousarray(grid[:, :, :, wc0 : wc0 + WS, :]).reshape(-1, C)
        in_maps.append({
            "u": np.ascontiguousarray(u[qi]),
            "grid": gs,
            "wofs": np.asarray([wc0], dtype=np.float32),
        })
    res = run_bass_kernel_spmd(nc, in_maps, list(range(NCORES)))
    out_sorted = np.concatenate(
        [res.results[c]["out"] for c in range(NCORES)], axis=0
    ).astype(np.float32)
    out = np.empty((B, C), dtype=np.float32)
    out[order] = out_sorted
    return out


if __name__ == "__main__":
    rng = np.random.default_rng(0)
    u = rng.random((B, 4), dtype=np.float32)
    grid = rng.standard_normal((*SIZES, C), dtype=np.float32)
    out = kernel(u, grid)
    print(out.shape, out.dtype)
